# revision 10
# baseline (speedup 1.0000x reference)
"""Vocab-parallel projection + cross-entropy loss kernel for TRN2 (8 NeuronCores).

Problem: x [2,2048,2048] f32, y [2,2048] int64, W [128000,2048] f32
  loss = mean_n( logsumexp_v(x_n . W_v) - x_n . W_{y_n} )

Sharding: W's vocab dim split 8 ways (16000 rows/core). Each core computes
  out_s[n] = sum_{v in shard} exp(logit[n, v])     (no max subtraction; logits ~ N(0, 1/3))
  out_t[n] = (y_n in shard) ? logit[n, y_n] : 0
Host combine: loss = mean(log(sum_i out_s_i) - sum_i out_t_i).

Per-core device kernel:
  - cast x, W_shard f32->bf16 through SBUF (HWDGE loads, DVE cast, HWDGE
    stores) -- keeps the SWDGE (gpsimd) queue free for the label gathers
  - XBAR transpose-load x^T (SBUF-resident, [128h x 16k x 4096tok] bf16)
  - per vocab tile (512): transpose-load W^T slab, 16 bf16 matmuls per
    128-token block accumulating [128tok x 512v] logits in PSUM,
    then one ScalarE Exp with accum_out -> per-(block,tile) partial sums
  - true logits: indirect-DMA gather of W[y_n] rows (f32) + multiply and
    reduce on VectorE, masked by validity
"""

import numpy as np

B, S, H, V = 2, 2048, 2048, 128000
N_CORES = 8
N_TOK = B * S                 # 4096
V_SHARD = V // N_CORES        # 16000
P = 128
V_TILE = 512                  # one PSUM bank of f32

_KERNEL_CACHE = {}


def _build(n_tok, h, vsh, debug=False, do_true=True, do_main=True):
    """Build + compile the single-core SPMD Bass program."""
    import concourse.bass as bass
    import concourse.mybir as mybir
    import concourse.tile as tile
    from concourse import bacc

    kt = h // P                       # k-tiles over hidden dim
    n_tb = n_tok // P                 # token blocks
    h2 = h // 2                       # half-row chunk for cast staging
    # vocab tiles: V_TILE plus remainder (must be multiple of 16 for XBAR)
    v_sizes = [V_TILE] * (vsh // V_TILE)
    if vsh % V_TILE:
        v_sizes.append(vsh % V_TILE)
    n_vt = len(v_sizes)

    nc = bacc.Bacc("TRN2", target_bir_lowering=False, debug=debug)
    f32 = mybir.dt.float32
    bf16 = mybir.dt.bfloat16

    x_in = nc.dram_tensor("x", [n_tok, h], f32, kind="ExternalInput")
    w_in = nc.dram_tensor("w", [vsh, h], f32, kind="ExternalInput")
    sel_in = nc.dram_tensor("sel", [n_tok], mybir.dt.int32, kind="ExternalInput")
    valid_in = nc.dram_tensor("valid", [n_tok], f32, kind="ExternalInput")
    out_s = nc.dram_tensor("out_s", [n_tok], f32, kind="ExternalOutput")
    out_t = nc.dram_tensor("out_t", [n_tok], f32, kind="ExternalOutput")

    xb = nc.dram_tensor("xb", [n_tok, h], bf16)      # bf16 copy of x
    wb = nc.dram_tensor("wb", [vsh, h], bf16)        # bf16 copy of W shard

    def cast_rows(dst, src, r0, nrows):
        """f32 DRAM rows -> bf16 DRAM rows via SBUF, in half-row chunks."""
        for cc in range(2):
            stage = stpool.tile([P, h2], f32, tag="stage")
            nc.scalar.dma_start(
                stage[:nrows], src[r0 : r0 + nrows, cc * h2 : (cc + 1) * h2]
            )
            cast = ctpool.tile([P, h2], bf16, tag="cast")
            nc.vector.tensor_copy(out=cast[:nrows], in_=stage[:nrows])
            nc.scalar.dma_start(
                dst[r0 : r0 + nrows, cc * h2 : (cc + 1) * h2], cast[:nrows]
            )

    with tile.TileContext(nc) as tc:
        with (
            tc.tile_pool(name="const", bufs=1) as cpool,
            tc.tile_pool(name="wslab", bufs=2) as wpool,
            tc.tile_pool(name="psum", bufs=8, space="PSUM") as ppool,
            tc.tile_pool(name="gath", bufs=2) as gpool,
            tc.tile_pool(name="xrow", bufs=2) as xpool,
            tc.tile_pool(name="junk", bufs=1) as jpool,
            tc.tile_pool(name="stage", bufs=2) as stpool,
            tc.tile_pool(name="castp", bufs=2) as ctpool,
        ):
            # ---- persistent SBUF tensors ----
            xT = cpool.tile([P, kt, n_tok], bf16, tag="xT")
            sacc = cpool.tile([P, n_tb, n_vt], f32, tag="sacc")
            tacc = cpool.tile([P, n_tb], f32, tag="tacc")
            tp2 = cpool.tile([P, n_tb], f32, tag="tp2")
            sel_sb = cpool.tile([P, n_tb], mybir.dt.int32, tag="sel")
            valid_sb = cpool.tile([P, n_tb], f32, tag="valid")
            s2 = cpool.tile([P, n_tb], f32, tag="s2")

            # load per-token metadata: token n = tb*128 + p  ->  [p, tb]
            nc.sync.dma_start(sel_sb[:], sel_in[:].rearrange("(a b) -> b a", b=P))
            nc.sync.dma_start(valid_sb[:], valid_in[:].rearrange("(a b) -> b a", b=P))

            # ---- phase T: true logits (independent of main loop; f32 W rows) ----
            for tb in range(n_tb if do_true else 0):
                wg = gpool.tile([P, h], f32, tag="wg")
                nc.gpsimd.indirect_dma_start(
                    out=wg[:],
                    out_offset=None,
                    in_=w_in[:],
                    in_offset=bass.IndirectOffsetOnAxis(ap=sel_sb[:, tb : tb + 1], axis=0),
                )
                for cc in range(2):
                    xf = xpool.tile([P, h2], f32, tag="xf")
                    nc.sync.dma_start(
                        xf[:], x_in[tb * P : (tb + 1) * P, cc * h2 : (cc + 1) * h2]
                    )
                    junk = jpool.tile([P, h2], f32, tag="junk")
                    nc.vector.tensor_tensor(
                        out=junk[:],
                        in0=xf[:],
                        in1=wg[:, cc * h2 : (cc + 1) * h2],
                        op=mybir.AluOpType.mult,
                    )
                    dst = tacc if cc == 0 else tp2
                    nc.vector.tensor_reduce(
                        out=dst[:, tb : tb + 1],
                        in_=junk[:],
                        axis=mybir.AxisListType.X,
                        op=mybir.AluOpType.add,
                    )
            if do_true:
                nc.vector.tensor_tensor(
                    out=tacc[:], in0=tacc[:], in1=tp2[:], op=mybir.AluOpType.add
                )
                # mask out tokens whose label is not in this shard
                nc.vector.tensor_tensor(
                    out=tacc[:], in0=tacc[:], in1=valid_sb[:], op=mybir.AluOpType.mult
                )
                nc.sync.dma_start(out_t[:].rearrange("(a b) -> b a", b=P), tacc[:])

            # ---- phase 0: x -> bf16 -> x^T (XBAR transpose loads) ----
            if do_main:
                for rb in range(n_tok // P):
                    cast_rows(xb, x_in, rb * P, P)
                for k in range(kt):
                    nc.sync.dma_start_transpose(xT[:, k, :], xb[:, k * P : (k + 1) * P])

            # ---- phase 1: main matmul + exp loop ----
            v0 = 0
            for vt, vsz in enumerate(v_sizes if do_main else []):
                for c0 in range(0, vsz, P):
                    cast_rows(wb, w_in, v0 + c0, min(P, vsz - c0))
                wslab = wpool.tile([P, kt, V_TILE], bf16, tag="wslab")
                for k in range(kt):
                    nc.sync.dma_start_transpose(
                        wslab[:, k, :vsz], wb[v0 : v0 + vsz, k * P : (k + 1) * P]
                    )
                for tb in range(n_tb):
                    psum = ppool.tile([P, V_TILE], f32, tag="psum")
                    for k in range(kt):
                        nc.tensor.matmul(
                            psum[:, :vsz],
                            lhsT=xT[:, k, tb * P : (tb + 1) * P],
                            rhs=wslab[:, k, :vsz],
                            start=(k == 0),
                            stop=(k == kt - 1),
                        )
                    # exp in place (PSUM), free-dim sum -> sacc[:, tb, vt]
                    nc.scalar.activation(
                        out=psum[:, :vsz],
                        in_=psum[:, :vsz],
                        func=mybir.ActivationFunctionType.Exp,
                        accum_out=sacc[:, tb, vt : vt + 1],
                    )
                v0 += vsz

            # ---- phase 2: finalize s ----
            if do_main:
                nc.vector.tensor_reduce(
                    out=s2[:], in_=sacc[:], axis=mybir.AxisListType.X, op=mybir.AluOpType.add
                )
                nc.sync.dma_start(out_s[:].rearrange("(a b) -> b a", b=P), s2[:])

    nc.compile()
    return nc


def _get_kernel(n_tok, h, vsh, debug=False):
    key = (n_tok, h, vsh, debug)
    if key not in _KERNEL_CACHE:
        _KERNEL_CACHE[key] = _build(n_tok, h, vsh, debug=debug)
    return _KERNEL_CACHE[key]


def make_in_maps(x, y, W, n_cores=N_CORES):
    """Shard full inputs into per-core input maps."""
    n_tok = x.shape[0] * x.shape[1] if x.ndim == 3 else x.shape[0]
    h = x.shape[-1]
    v = W.shape[0]
    vsh = v // n_cores
    xf = np.ascontiguousarray(x.reshape(n_tok, h), dtype=np.float32)
    yf = y.reshape(n_tok).astype(np.int64)
    in_maps = []
    for c in range(n_cores):
        lo, hi = c * vsh, (c + 1) * vsh
        owned = (yf >= lo) & (yf < hi)
        sel = np.where(owned, yf - lo, 0).astype(np.int32)
        valid = owned.astype(np.float32)
        in_maps.append(
            {
                "x": xf,
                "w": np.ascontiguousarray(W[lo:hi], dtype=np.float32),
                "sel": sel,
                "valid": valid,
            }
        )
    return in_maps


def combine(results):
    """Host-side unshard: reduce per-core partials to the scalar loss."""
    s = np.sum([r["out_s"].astype(np.float64) for r in results], axis=0)
    t = np.sum([r["out_t"].astype(np.float64) for r in results], axis=0)
    return np.float32(np.mean(np.log(s) - t))


def run_sharded(x, y, W, trace=False):
    from concourse.bass_utils import run_bass_kernel_spmd

    n_tok = x.reshape(-1, x.shape[-1]).shape[0]
    h = x.shape[-1]
    vsh = W.shape[0] // N_CORES
    nc = _get_kernel(n_tok, h, vsh)
    in_maps = make_in_maps(x, y, W)
    res = run_bass_kernel_spmd(nc, in_maps, list(range(N_CORES)), trace=trace)
    return res


def kernel(x, y, W):
    res = run_sharded(np.asarray(x), np.asarray(y), np.asarray(W))
    return combine(res.results)


# revision 11
# speedup vs baseline: 1.8687x; 1.8687x over previous
"""Vocab-parallel projection + cross-entropy loss kernel for TRN2 (8 NeuronCores).

Problem: x [2,2048,2048] f32, y [2,2048] int64, W [128000,2048] f32
  loss = mean_n( logsumexp_v(x_n . W_v) - x_n . W_{y_n} )

Sharding (8 cores):
  - W's vocab dim split 8 ways (16000 rows/core): each core computes
    out_s[n] = sum_{v in shard} exp(logit[n, v]) for all 4096 tokens.
    (No max subtraction needed: logits ~ N(0, 1/3).)
  - tokens split 8 ways for the true-logit term: core c receives
    xy = x rows and wy = W[y] rows for its 512 tokens and computes
    out_t[j] = xy[j] . wy[j] on VectorE.
Host combine: loss = mean(log(sum_i out_s_i) - concat_i out_t_i).

Per-core device kernel (fp8 path):
  - W shard: SWDGE cast-DMA f32->bf16 into DRAM, XBAR transpose-load
    [h x v] bf16 slabs, VectorE scale(x64)+cast to fp8e4
  - x: HWDGE load + VectorE cast to bf16 DRAM, XBAR transpose-load,
    VectorE scale(x32)+cast to fp8e4 (x^T resident in SBUF)
  - per vocab tile (512): 8 DoubleRow fp8 matmuls per 128-token block
    accumulate [128tok x 512v] logits*2048 in PSUM; one ScalarE Exp with
    scale=1/2048 and accum_out -> per-(block,tile) partial sums
"""

import numpy as np

B, S, H, V = 2, 2048, 2048, 128000
N_CORES = 8
N_TOK = B * S                 # 4096
V_SHARD = V // N_CORES        # 16000
TOK_SHARD = N_TOK // N_CORES  # 512
P = 128
V_TILE = 512                  # one PSUM bank of f32
X_SCALE = 32.0
W_SCALE = 64.0

_KERNEL_CACHE = {}


def _build(n_tok, h, vsh, tok_sh, use_fp8=True, debug=False, do_true=True, do_main=True):
    """Build + compile the single-core SPMD Bass program."""
    import concourse.mybir as mybir
    import concourse.tile as tile
    from concourse import bacc

    kt = h // P                       # k-tiles over hidden dim
    n_tb = n_tok // P                 # token blocks
    v_sizes = [V_TILE] * (vsh // V_TILE)
    if vsh % V_TILE:
        v_sizes.append(vsh % V_TILE)  # remainder must be multiple of 16 (XBAR)
    n_vt = len(v_sizes)
    descale = 1.0 / (X_SCALE * W_SCALE) if use_fp8 else 1.0

    nc = bacc.Bacc("TRN2", target_bir_lowering=False, debug=debug)
    f32 = mybir.dt.float32
    bf16 = mybir.dt.bfloat16
    fp8 = mybir.dt.float8e4
    mm_dt = fp8 if use_fp8 else bf16

    x_in = nc.dram_tensor("x", [n_tok, h], f32, kind="ExternalInput")
    w_in = nc.dram_tensor("w", [vsh, h], f32, kind="ExternalInput")
    xy_in = nc.dram_tensor("xy", [tok_sh, h], f32, kind="ExternalInput")
    wy_in = nc.dram_tensor("wy", [tok_sh, h], f32, kind="ExternalInput")
    out_s = nc.dram_tensor("out_s", [n_tok], f32, kind="ExternalOutput")
    out_t = nc.dram_tensor("out_t", [tok_sh], f32, kind="ExternalOutput")

    xb = nc.dram_tensor("xb", [n_tok, h], bf16)      # bf16 copy of x
    wb = nc.dram_tensor("wb", [vsh, h], bf16)        # bf16 copy of W shard

    with tile.TileContext(nc) as tc:
        with (
            tc.tile_pool(name="const", bufs=1) as cpool,
            tc.tile_pool(name="wslab", bufs=2) as wpool,
            tc.tile_pool(name="w8p", bufs=2) as w8pool,
            tc.tile_pool(name="psum", bufs=8, space="PSUM") as ppool,
            tc.tile_pool(name="gath", bufs=2) as gpool,
            tc.tile_pool(name="xrow", bufs=2) as xpool,
            tc.tile_pool(name="junk", bufs=1) as jpool,
            tc.tile_pool(name="stage", bufs=2) as stpool,
            tc.tile_pool(name="castp", bufs=2) as ctpool,
            tc.tile_pool(name="xtmp", bufs=2) as xtpool,
        ):
            # ---- persistent SBUF tensors ----
            xT = cpool.tile([P, kt, n_tok], mm_dt, tag="xT")
            sacc = cpool.tile([P, n_tb, n_vt], f32, tag="sacc")
            tacc = cpool.tile([P, tok_sh // P], f32, tag="tacc")
            s2 = cpool.tile([P, n_tb], f32, tag="s2")

            # ---- phase T: true logits for this core's token slice ----
            for c in range(tok_sh // P if do_true else 0):
                wy = gpool.tile([P, h], f32, tag="wy")
                nc.sync.dma_start(wy[:], wy_in[c * P : (c + 1) * P, :])
                xf = xpool.tile([P, h], f32, tag="xf")
                nc.sync.dma_start(xf[:], xy_in[c * P : (c + 1) * P, :])
                junk = jpool.tile([P, h], f32, tag="junk")
                nc.vector.tensor_tensor(
                    out=junk[:], in0=xf[:], in1=wy[:], op=mybir.AluOpType.mult
                )
                nc.vector.tensor_reduce(
                    out=tacc[:, c : c + 1],
                    in_=junk[:],
                    axis=mybir.AxisListType.X,
                    op=mybir.AluOpType.add,
                )
            if do_true:
                nc.sync.dma_start(out_t[:].rearrange("(a b) -> b a", b=P), tacc[:])

            if do_main:
                # ---- phase 0: x -> bf16 (HWDGE+DVE) -> x^T -> mm dtype ----
                for rb in range(n_tok // P):
                    stage = stpool.tile([P, h], f32, tag="stage")
                    nc.scalar.dma_start(stage[:], x_in[rb * P : (rb + 1) * P, :])
                    cast = ctpool.tile([P, h], bf16, tag="cast")
                    nc.vector.tensor_copy(out=cast[:], in_=stage[:])
                    nc.scalar.dma_start(xb[rb * P : (rb + 1) * P, :], cast[:])
                n_half = n_tok // 2
                for half in range(2):
                    for k in range(kt):
                        if use_fp8:
                            xtmp = xtpool.tile([P, n_half], bf16, tag="xtmp")
                            nc.sync.dma_start_transpose(
                                xtmp[:],
                                xb[half * n_half : (half + 1) * n_half, k * P : (k + 1) * P],
                            )
                            nc.vector.tensor_scalar_mul(
                                xT[:, k, half * n_half : (half + 1) * n_half],
                                xtmp[:],
                                X_SCALE,
                            )
                        else:
                            nc.sync.dma_start_transpose(
                                xT[:, k, half * n_half : (half + 1) * n_half],
                                xb[half * n_half : (half + 1) * n_half, k * P : (k + 1) * P],
                            )

            # ---- phase 1: main matmul + exp loop ----
            v0 = 0
            for vt, vsz in enumerate(v_sizes if do_main else []):
                # W rows -> bf16 via SWDGE cast-DMA (DRAM->DRAM)
                nc.gpsimd.dma_start(wb[v0 : v0 + vsz, :], w_in[v0 : v0 + vsz, :])
                wslab = wpool.tile([P, kt, V_TILE], bf16, tag="wslab")
                for k in range(kt):
                    nc.sync.dma_start_transpose(
                        wslab[:, k, :vsz], wb[v0 : v0 + vsz, k * P : (k + 1) * P]
                    )
                if use_fp8:
                    w8 = w8pool.tile([P, kt, V_TILE], fp8, tag="w8")
                    nc.vector.tensor_scalar_mul(w8[:], wslab[:], W_SCALE)
                    rhs_slab = w8
                else:
                    rhs_slab = wslab
                for tb in range(n_tb):
                    psum = ppool.tile([P, V_TILE], f32, tag="psum")
                    if use_fp8:
                        for kk in range(0, kt, 2):
                            nc.tensor.matmul(
                                psum[:, :vsz],
                                lhsT=xT[:, kk : kk + 2, tb * P : (tb + 1) * P],
                                rhs=rhs_slab[:, kk : kk + 2, :vsz],
                                start=(kk == 0),
                                stop=(kk == kt - 2),
                                perf_mode=mybir.MatmulPerfMode.DoubleRow,
                            )
                    else:
                        for k in range(kt):
                            nc.tensor.matmul(
                                psum[:, :vsz],
                                lhsT=xT[:, k, tb * P : (tb + 1) * P],
                                rhs=rhs_slab[:, k, :vsz],
                                start=(k == 0),
                                stop=(k == kt - 1),
                            )
                    # exp(descale * psum) in place, free-dim sum -> sacc
                    nc.scalar.activation(
                        out=psum[:, :vsz],
                        in_=psum[:, :vsz],
                        func=mybir.ActivationFunctionType.Exp,
                        scale=descale,
                        accum_out=sacc[:, tb, vt : vt + 1],
                    )
                v0 += vsz

            # ---- phase 2: finalize s ----
            if do_main:
                nc.vector.tensor_reduce(
                    out=s2[:], in_=sacc[:], axis=mybir.AxisListType.X, op=mybir.AluOpType.add
                )
                nc.sync.dma_start(out_s[:].rearrange("(a b) -> b a", b=P), s2[:])

    nc.compile()
    return nc


def _get_kernel(n_tok, h, vsh, tok_sh):
    key = (n_tok, h, vsh, tok_sh)
    if key not in _KERNEL_CACHE:
        _KERNEL_CACHE[key] = _build(n_tok, h, vsh, tok_sh)
    return _KERNEL_CACHE[key]


def make_in_maps(x, y, W, n_cores=N_CORES):
    """Shard full inputs into per-core input maps."""
    n_tok = x.reshape(-1, x.shape[-1]).shape[0]
    h = x.shape[-1]
    v = W.shape[0]
    vsh = v // n_cores
    tok_sh = n_tok // n_cores
    xf = np.ascontiguousarray(x.reshape(n_tok, h), dtype=np.float32)
    yf = y.reshape(n_tok)
    wy_full = np.ascontiguousarray(W[yf], dtype=np.float32)  # [n_tok, h]
    in_maps = []
    for c in range(n_cores):
        lo, hi = c * vsh, (c + 1) * vsh
        t0, t1 = c * tok_sh, (c + 1) * tok_sh
        in_maps.append(
            {
                "x": xf,
                "w": np.ascontiguousarray(W[lo:hi], dtype=np.float32),
                "xy": np.ascontiguousarray(xf[t0:t1]),
                "wy": np.ascontiguousarray(wy_full[t0:t1]),
            }
        )
    return in_maps


def combine(results):
    """Host-side unshard: reduce per-core partials to the scalar loss."""
    s = np.sum([r["out_s"].astype(np.float64) for r in results], axis=0)
    t = np.concatenate([r["out_t"].astype(np.float64) for r in results])
    return np.float32(np.mean(np.log(s) - t))


def run_sharded(x, y, W, trace=False):
    from concourse.bass_utils import run_bass_kernel_spmd

    n_tok = x.reshape(-1, x.shape[-1]).shape[0]
    h = x.shape[-1]
    vsh = W.shape[0] // N_CORES
    nc = _get_kernel(n_tok, h, vsh, n_tok // N_CORES)
    in_maps = make_in_maps(x, y, W)
    res = run_bass_kernel_spmd(nc, in_maps, list(range(N_CORES)), trace=trace)
    return res


def kernel(x, y, W):
    res = run_sharded(np.asarray(x), np.asarray(y), np.asarray(W))
    return combine(res.results)
